# revision 12
# baseline (speedup 1.0000x reference)
"""Trainium2 Bass kernel for nn_DRNLayer (8-core n_upper-sharded).

out[i,j,l] = softmax_l( sum_k log( sum_m exp(w[j,k]*logD[m,l]) * P[i,k,m] ) + B[j,l] )

Sharding: n_upper (j) split 8 ways, 16 j per core; softmax axis (q_up) is local,
so no collectives; host concatenates per-core outputs.

Per-core dataflow:
  - PE builds the exp arguments w[j,k]*logD[m,l] as a K=18 split-bf16 matmul
    against host-precomputed rank-3 factors (logD[m,l] = -(mv-lv)^2 is rank 3
    in m x l), writing PSUM.
  - ACT exps PSUM -> fp32r T tiles [128=(khalf,m), (jh,j,l)].
  - PE computes Pw for two k's at once with a block-diagonal fp32r lhsT
    [[P_k^T, 0], [0, P_k'^T]] (K=128, dst partitions 0..127, N=512).
  - DVE runs a product chain over groups of 8 k-pairs (PSUM x SBUF), ACT
    takes one Ln per group, Pool accumulates the group logs.
  - PE folds the cross-k-half partition sum + bias via small fp32 matmuls,
    then a local softmax over l and DMA out.
"""

import sys

sys.path.insert(0, "/opt/trn_rl_repo")

from contextlib import ExitStack

import ml_dtypes
import numpy as np

import concourse.bacc as bacc
import concourse.bass as bass
import concourse.mybir as mybir
from concourse.bass_utils import run_bass_kernel_spmd
from concourse.tile import TileContext

F32 = mybir.dt.float32
F32R = mybir.dt.float32r
BF16 = mybir.dt.bfloat16
I32 = mybir.dt.int32
AF = mybir.ActivationFunctionType
ALU = mybir.AluOpType
BF16_NP = ml_dtypes.bfloat16

N_CORES = 8
BATCH = 64  # i
NJ = 16  # j per core
NK = 128  # n_lower (k)
Q = 64  # q_upper == q_lower (l, m)
NKP = 64  # k-pairs: partition half 0 handles k=kp, half 1 handles k=kp+64
GS = 8  # k-pairs per product group
KCH = 8  # k-pairs per H-chunk DMA
ACT_TABLE_LN_EXP = 6  # act_info.json index of natural_log_exp_and_others

_NC = None
LAST_RESULTS = None


def _build():
    nc = bacc.Bacc()
    P_d = nc.declare_dram_parameter("PTH", [2, Q, NKP, BATCH], F32R, isOutput=False)
    F_d = nc.declare_dram_parameter("F128", [128, 128], BF16, isOutput=False)
    H_d = nc.declare_dram_parameter("H18", [18, NKP, 2, 512], BF16, isOutput=False)
    b_d = nc.declare_dram_parameter("Bflat", [1, NJ * Q], F32, isOutput=False)
    o_d = nc.declare_dram_parameter("out", [BATCH, NJ, Q], F32, isOutput=True)

    with TileContext(nc) as tc, ExitStack() as ctx:
        # keep Exp+Ln resident in one ACT table for the whole kernel
        nc.scalar.add_instruction(
            mybir.InstLoadActFuncSet(
                name=nc.get_next_instruction_name(),
                ins=[],
                outs=[],
                act_func_set_id=ACT_TABLE_LN_EXP,
            )
        )

        consts = ctx.enter_context(tc.tile_pool(name="consts", bufs=1))
        ptbp = ctx.enter_context(tc.tile_pool(name="ptb", bufs=1))
        hpool = ctx.enter_context(tc.tile_pool(name="hp", bufs=2))
        tpool = ctx.enter_context(tc.tile_pool(name="tpair", bufs=3))
        apool = ctx.enter_context(tc.tile_pool(name="acc", bufs=2))
        gpool = ctx.enter_context(tc.tile_pool(name="glog", bufs=2))
        lpool = ctx.enter_context(tc.tile_pool(name="lsum", bufs=1))
        spool = ctx.enter_context(tc.tile_pool(name="smax", bufs=1))

        # ---------------- constants / inputs ----------------
        F128 = consts.tile([128, 128], BF16)
        nc.sync.dma_start(out=F128, in_=F_d[:, :])
        Brow = consts.tile([1, NJ * Q], F32)
        nc.sync.dma_start(out=Brow, in_=b_d[:, :])

        # it[p, l] = l - (p % 64); comb[p, i] = (p % 64 == i)
        it = consts.tile([128, Q], I32)
        nc.gpsimd.iota(it, pattern=[[1, Q]], base=0, channel_multiplier=-1)
        nc.vector.tensor_scalar_add(it[64:128, :], it[64:128, :], 64)
        comb = consts.tile([128, BATCH], F32)
        nc.vector.tensor_scalar(comb, it, 0, None, ALU.is_equal)
        ones64 = consts.tile([1, BATCH], F32)
        nc.vector.memset(ones64, 1.0)

        # ---------------- phase 1: load host-transposed P ----------------
        # PTB[64*c+m, kp, 64*c+i] = P[i, kp + 64*c, m], zero off-diagonal
        PTB = ptbp.tile([128, NKP, 128], F32R)
        nc.gpsimd.memset(PTB[0:64, :, 64:128].bitcast(F32), 0.0)
        nc.gpsimd.memset(PTB[64:128, :, 0:64].bitcast(F32), 0.0)
        nc.sync.dma_start(out=PTB[0:64, :, 0:64], in_=P_d[0])
        nc.sync.dma_start(out=PTB[64:128, :, 64:128], in_=P_d[1])

        # running log-sum accumulator (SBUF, fp32)
        logsum = lpool.tile([128, NJ * Q], F32)

        # ---------------- phase 2: main loop over k-pairs ----------------
        with tc.tile_pool(name="argps", bufs=2, space="PSUM") as ps_arg, tc.tile_pool(
            name="pwps", bufs=2, space="PSUM"
        ) as ps_pw:
            acc = None
            for kp in range(NKP):
                if kp % KCH == 0:
                    Hch = hpool.tile([128, KCH, 2, 512], BF16, tag="hch")
                    for b in range(7):
                        nc.sync.dma_start(
                            out=Hch[b * 18 : (b + 1) * 18],
                            in_=H_d[:, kp : kp + KCH, :, :],
                        )
                    nc.sync.dma_start(
                        out=Hch[126:128], in_=H_d[0:2, kp : kp + KCH, :, :]
                    )
                kc = kp % KCH
                g = kp // GS

                argp = ps_arg.tile([128, 2, 512], F32)  # 2 banks
                for jh in range(2):
                    nc.tensor.matmul(
                        out=argp[:, jh, :],
                        lhsT=F128,
                        rhs=Hch[:, kc, jh, :],
                        start=True,
                        stop=True,
                    )
                tp = tpool.tile([128, 2, 512], F32R, tag="texp")
                nc.scalar.activation(
                    out=tp.rearrange("p a b -> p (a b)"),
                    in_=argp.rearrange("p a b -> p (a b)"),
                    func=AF.Exp,
                )
                pw = ps_pw.tile([128, 2, 512], F32)  # 2 banks
                for jh in range(2):
                    nc.tensor.matmul(
                        out=pw[:, jh, :],
                        lhsT=PTB[:, kp, :],
                        rhs=tp[:, jh, :],
                        start=True,
                        stop=True,
                    )
                pw_flat = pw.rearrange("p a b -> p (a b)")
                if kp % GS == 0:
                    acc = apool.tile([128, NJ * Q], F32)
                    nc.vector.tensor_copy(out=acc, in_=pw_flat)
                else:
                    nc.vector.tensor_tensor(
                        out=acc, in0=pw_flat, in1=acc, op=ALU.mult
                    )
                if kp % GS == GS - 1:
                    if g == 0:
                        nc.scalar.activation(out=logsum, in_=acc, func=AF.Ln)
                    else:
                        gl = gpool.tile([128, NJ * Q], F32, tag="gl")
                        nc.scalar.activation(out=gl, in_=acc, func=AF.Ln)
                        nc.gpsimd.tensor_tensor(
                            out=logsum, in0=gl, in1=logsum, op=ALU.add
                        )

        # ---------------- phase 3: combine + bias + softmax ----------------
        with tc.tile_pool(name="lg", bufs=1, space="PSUM") as ps_lg:
            logits = ps_lg.tile([BATCH, 2, 512], F32)
            for jh in range(2):
                nc.tensor.matmul(
                    out=logits[:, jh, :],
                    lhsT=comb,
                    rhs=logsum[:, jh * 512 : (jh + 1) * 512],
                    start=True,
                    stop=False,
                    skip_group_check=True,
                )
                nc.tensor.matmul(
                    out=logits[:, jh, :],
                    lhsT=ones64,
                    rhs=Brow[:, jh * 512 : (jh + 1) * 512],
                    start=False,
                    stop=True,
                    skip_group_check=True,
                )

            lg_v = logits.rearrange("p a (j l) -> p (a j) l", l=Q)  # [64, 16, 64]
            mx = spool.tile([BATCH, NJ], F32)
            nc.vector.tensor_reduce(mx, lg_v, axis=mybir.AxisListType.X, op=ALU.max)
            em = spool.tile([BATCH, NJ, Q], F32)
            nc.vector.tensor_tensor(
                out=em,
                in0=lg_v,
                in1=mx.unsqueeze(2).broadcast_to([BATCH, NJ, Q]),
                op=ALU.subtract,
            )
            nc.scalar.activation(out=em, in_=em, func=AF.Exp)
            sm = spool.tile([BATCH, NJ], F32)
            nc.vector.tensor_reduce(sm, em, axis=mybir.AxisListType.X, op=ALU.add)
            rec = spool.tile([BATCH, NJ], F32)
            nc.vector.reciprocal(rec, sm)
            oute = spool.tile([BATCH, NJ, Q], F32)
            nc.gpsimd.tensor_tensor(
                out=oute,
                in0=em,
                in1=rec.unsqueeze(2).broadcast_to([BATCH, NJ, Q]),
                op=ALU.mult,
            )
            nc.sync.dma_start(out=o_d[:, :, :], in_=oute)

    nc.compile()
    return nc


def _bf16_split(x):
    hi = x.astype(BF16_NP)
    lo = (x - hi.astype(np.float32)).astype(BF16_NP)
    return hi, lo


def _host_factors(wsl):
    """F18 [18, 128] bf16 and H18 [18, NKP, 2, 512] bf16 for one core.

    arg[j,k,m,l] = w[j,k] * (-(mv-lv)^2) = sum_r f_r(mv) * h_r(w, lv)
      f = [mv^2, mv, 1],  h = [-w, 2*w*lv, -w*lv^2]
    K rows ordered (t, c, r): t = split term (Fhi*Hhi, Flo*Hhi, Fhi*Hlo),
    c = k-half (F zeroed outside its c partition block), r = rank.
    """
    mv = (np.arange(Q, dtype=np.float32) / Q).astype(np.float32)
    lv = mv
    f = np.stack([mv * mv, mv, np.ones(Q, dtype=np.float32)]) / 7.0  # [3, 64]
    fhi, flo = _bf16_split(f)
    F18 = np.zeros((3, 2, 3, 128), dtype=BF16_NP)
    for c in range(2):
        sl = slice(c * 64, (c + 1) * 64)
        F18[0, c, :, sl] = fhi
        F18[1, c, :, sl] = flo
        F18[2, c, :, sl] = fhi
    F128 = np.zeros((128, 128), dtype=BF16_NP)
    F128[0:126] = np.tile(F18.reshape(18, 128), (7, 1))

    # h[r, c, j, kp, l] with w = wsl[j, kp + 64 c]
    wz = np.stack([wsl[:, 0:NKP], wsl[:, NKP:]], axis=0)  # [c, j, kp]
    h = np.empty((3, 2, NJ, NKP, Q), dtype=np.float32)
    h[0] = -wz[..., None] * np.ones(Q, dtype=np.float32)
    h[1] = 2.0 * wz[..., None] * lv
    h[2] = -wz[..., None] * (lv * lv)
    hhi, hlo = _bf16_split(h)
    H18 = np.zeros((3, 2, 3, NKP, 2, 8, Q), dtype=BF16_NP)
    for t, hv in ((0, hhi), (1, hhi), (2, hlo)):
        # hv [r, c, j, kp, l] -> [c, r, kp, jh, j8, l]
        H18[t] = (
            hv.reshape(3, 2, 2, 8, NKP, Q)
            .transpose(1, 0, 4, 2, 3, 5)
            .astype(BF16_NP)
        )
    return (
        np.ascontiguousarray(F128),
        np.ascontiguousarray(H18.reshape(18, NKP, 2, 512)),
    )


def kernel(P, weight, bias_abs, bias_q, lambda_abs, lambda_q):
    global _NC, LAST_RESULTS
    P = np.asarray(P, dtype=np.float32)
    PTH = np.ascontiguousarray(P.reshape(BATCH, 2, NKP, Q).transpose(1, 3, 2, 0))
    weight = np.asarray(weight, dtype=np.float32)
    bias_abs = np.asarray(bias_abs, dtype=np.float32)
    bias_q = np.asarray(bias_q, dtype=np.float32)
    lambda_abs = np.asarray(lambda_abs, dtype=np.float32)
    lambda_q = np.asarray(lambda_q, dtype=np.float32)

    if _NC is None:
        _NC = _build()

    s = (np.arange(Q, dtype=np.float32) / Q)[None, :]  # [1, 64]
    in_maps = []
    for c in range(N_CORES):
        jsl = slice(c * NJ, (c + 1) * NJ)
        F128, H18 = _host_factors(weight[jsl, :])
        Bm = -bias_q[jsl] * (s - lambda_q[jsl]) ** 2 - bias_abs[jsl] * np.abs(
            s - lambda_abs[jsl]
        )  # [16, 64]
        in_maps.append(
            {
                "PTH": PTH,
                "F128": F128,
                "H18": H18,
                "Bflat": np.ascontiguousarray(Bm.reshape(1, NJ * Q)),
            }
        )

    LAST_RESULTS = run_bass_kernel_spmd(_NC, in_maps, list(range(N_CORES)))
    return np.concatenate(
        [LAST_RESULTS.results[c]["out"] for c in range(N_CORES)], axis=1
    )


# revision 14
# speedup vs baseline: 1.0217x; 1.0217x over previous
"""Trainium2 Bass kernel for nn_DRNLayer (8-core n_upper-sharded).

out[i,j,l] = softmax_l( sum_k log( sum_m exp(w[j,k]*logD[m,l]) * P[i,k,m] ) + B[j,l] )

Sharding: n_upper (j) split 8 ways, 16 j per core; softmax axis (q_up) is local,
so no collectives; host concatenates per-core outputs.

Per-core dataflow:
  - PE builds the exp arguments w[j,k]*logD[m,l] as a K=18 split-bf16 matmul
    against host-precomputed rank-3 factors (logD[m,l] = -(mv-lv)^2 is rank 3
    in m x l), writing PSUM.
  - ACT exps PSUM -> fp32r T tiles [128=(khalf,m), (jh,j,l)].
  - PE computes Pw for two k's at once with a block-diagonal fp32r lhsT
    [[P_k^T, 0], [0, P_k'^T]] (K=128, dst partitions 0..127, N=512).
  - DVE runs a product chain over groups of 8 k-pairs (PSUM x SBUF), ACT
    takes one Ln per group, Pool accumulates the group logs.
  - PE folds the cross-k-half partition sum + bias via small fp32 matmuls,
    then a local softmax over l and DMA out.
"""

import sys

sys.path.insert(0, "/opt/trn_rl_repo")

from contextlib import ExitStack

import ml_dtypes
import numpy as np

import concourse.bacc as bacc
import concourse.bass as bass
import concourse.mybir as mybir
from concourse.bass_utils import run_bass_kernel_spmd
from concourse.tile import TileContext

F32 = mybir.dt.float32
F32R = mybir.dt.float32r
BF16 = mybir.dt.bfloat16
I32 = mybir.dt.int32
AF = mybir.ActivationFunctionType
ALU = mybir.AluOpType
BF16_NP = ml_dtypes.bfloat16

N_CORES = 8
BATCH = 64  # i
NJ = 16  # j per core
NK = 128  # n_lower (k)
Q = 64  # q_upper == q_lower (l, m)
NKP = 64  # k-pairs: partition half 0 handles k=kp, half 1 handles k=kp+64
GS = 8  # k-pairs per product group
KCH = 8  # k-pairs per H-chunk DMA
ACT_TABLE_LN_EXP = 6  # act_info.json index of natural_log_exp_and_others

_NC = None
LAST_RESULTS = None


def _build():
    nc = bacc.Bacc()
    P_d = nc.declare_dram_parameter("PTH", [2, Q, NKP, BATCH], F32R, isOutput=False)
    F_d = nc.declare_dram_parameter("F128", [128, 128], BF16, isOutput=False)
    H_d = nc.declare_dram_parameter("H18", [18, NKP, 2, 512], BF16, isOutput=False)
    b_d = nc.declare_dram_parameter("Bflat", [1, NJ * Q], F32, isOutput=False)
    o_d = nc.declare_dram_parameter("out", [BATCH, NJ, Q], F32, isOutput=True)
    Hrep_d = nc.dram_tensor("Hrep", [128, NKP, 2, 512], BF16)

    with TileContext(nc) as tc, ExitStack() as ctx:
        # keep Exp+Ln resident in one ACT table for the whole kernel
        nc.scalar.add_instruction(
            mybir.InstLoadActFuncSet(
                name=nc.get_next_instruction_name(),
                ins=[],
                outs=[],
                act_func_set_id=ACT_TABLE_LN_EXP,
            )
        )

        consts = ctx.enter_context(tc.tile_pool(name="consts", bufs=1))
        ptbp = ctx.enter_context(tc.tile_pool(name="ptb", bufs=1))
        hpool = ctx.enter_context(tc.tile_pool(name="hp", bufs=2))
        tpool = ctx.enter_context(tc.tile_pool(name="tpair", bufs=4))
        apool = ctx.enter_context(tc.tile_pool(name="acc", bufs=2))
        gpool = ctx.enter_context(tc.tile_pool(name="glog", bufs=2))
        lpool = ctx.enter_context(tc.tile_pool(name="lsum", bufs=1))
        spool = ctx.enter_context(tc.tile_pool(name="smax", bufs=1))

        # ---------------- constants / inputs ----------------
        F128 = consts.tile([128, 128], BF16)
        nc.sync.dma_start(out=F128, in_=F_d[:, :])
        for b in range(7):
            nc.sync.dma_start(out=Hrep_d[b * 18 : (b + 1) * 18], in_=H_d[:, :, :, :])
        nc.sync.dma_start(out=Hrep_d[126:128], in_=H_d[0:2, :, :, :])
        Brow = consts.tile([1, NJ * Q], F32)
        nc.sync.dma_start(out=Brow, in_=b_d[:, :])

        # it[p, l] = l - (p % 64); comb[p, i] = (p % 64 == i)
        it = consts.tile([128, Q], I32)
        nc.gpsimd.iota(it, pattern=[[1, Q]], base=0, channel_multiplier=-1)
        nc.vector.tensor_scalar_add(it[64:128, :], it[64:128, :], 64)
        comb = consts.tile([128, BATCH], F32)
        nc.vector.tensor_scalar(comb, it, 0, None, ALU.is_equal)
        ones64 = consts.tile([1, BATCH], F32)
        nc.vector.memset(ones64, 1.0)

        # ---------------- phase 1: load host-transposed P ----------------
        # PTB[64*c+m, kp, 64*c+i] = P[i, kp + 64*c, m], zero off-diagonal
        PTB = ptbp.tile([128, NKP, 128], F32R)
        nc.gpsimd.memset(PTB[0:64, :, 64:128].bitcast(F32), 0.0)
        nc.gpsimd.memset(PTB[64:128, :, 0:64].bitcast(F32), 0.0)
        nc.sync.dma_start(out=PTB[0:64, :, 0:64], in_=P_d[0])
        nc.sync.dma_start(out=PTB[64:128, :, 64:128], in_=P_d[1])

        # running log-sum accumulator (SBUF, fp32)
        logsum = lpool.tile([128, NJ * Q], F32)

        # ---------------- phase 2: main loop over k-pairs ----------------
        with tc.tile_pool(name="argps", bufs=2, space="PSUM") as ps_arg, tc.tile_pool(
            name="pwps", bufs=2, space="PSUM"
        ) as ps_pw:
            acc = None
            for kp in range(NKP):
                if kp % KCH == 0:
                    Hch = hpool.tile([128, KCH, 2, 512], BF16, tag="hch")
                    nc.sync.dma_start(out=Hch, in_=Hrep_d[:, kp : kp + KCH, :, :])
                kc = kp % KCH
                g = kp // GS

                argp = ps_arg.tile([128, 2, 512], F32)  # 2 banks
                for jh in range(2):
                    nc.tensor.matmul(
                        out=argp[:, jh, :],
                        lhsT=F128,
                        rhs=Hch[:, kc, jh, :],
                        start=True,
                        stop=True,
                    )
                tp = tpool.tile([128, 2, 512], F32R, tag="texp")
                nc.scalar.activation(
                    out=tp.rearrange("p a b -> p (a b)"),
                    in_=argp.rearrange("p a b -> p (a b)"),
                    func=AF.Exp,
                )
                pw = ps_pw.tile([128, 2, 512], F32)  # 2 banks
                for jh in range(2):
                    nc.tensor.matmul(
                        out=pw[:, jh, :],
                        lhsT=PTB[:, kp, :],
                        rhs=tp[:, jh, :],
                        start=True,
                        stop=True,
                    )
                pw_flat = pw.rearrange("p a b -> p (a b)")
                if kp % GS == 0:
                    acc = apool.tile([128, NJ * Q], F32)
                    nc.vector.tensor_copy(out=acc, in_=pw_flat)
                else:
                    nc.vector.tensor_tensor(
                        out=acc, in0=pw_flat, in1=acc, op=ALU.mult
                    )
                if kp % GS == GS - 1:
                    if g == 0:
                        nc.scalar.activation(out=logsum, in_=acc, func=AF.Ln)
                    else:
                        gl = gpool.tile([128, NJ * Q], F32, tag="gl")
                        nc.scalar.activation(out=gl, in_=acc, func=AF.Ln)
                        nc.gpsimd.tensor_tensor(
                            out=logsum, in0=gl, in1=logsum, op=ALU.add
                        )

        # ---------------- phase 3: combine + bias + softmax ----------------
        with tc.tile_pool(name="lg", bufs=1, space="PSUM") as ps_lg:
            logits = ps_lg.tile([BATCH, 2, 512], F32)
            for jh in range(2):
                nc.tensor.matmul(
                    out=logits[:, jh, :],
                    lhsT=comb,
                    rhs=logsum[:, jh * 512 : (jh + 1) * 512],
                    start=True,
                    stop=False,
                    skip_group_check=True,
                )
                nc.tensor.matmul(
                    out=logits[:, jh, :],
                    lhsT=ones64,
                    rhs=Brow[:, jh * 512 : (jh + 1) * 512],
                    start=False,
                    stop=True,
                    skip_group_check=True,
                )

            lg_v = logits.rearrange("p a (j l) -> p (a j) l", l=Q)  # [64, 16, 64]
            mx = spool.tile([BATCH, NJ], F32)
            nc.vector.tensor_reduce(mx, lg_v, axis=mybir.AxisListType.X, op=ALU.max)
            em = spool.tile([BATCH, NJ, Q], F32)
            nc.vector.tensor_tensor(
                out=em,
                in0=lg_v,
                in1=mx.unsqueeze(2).broadcast_to([BATCH, NJ, Q]),
                op=ALU.subtract,
            )
            nc.scalar.activation(out=em, in_=em, func=AF.Exp)
            sm = spool.tile([BATCH, NJ], F32)
            nc.vector.tensor_reduce(sm, em, axis=mybir.AxisListType.X, op=ALU.add)
            rec = spool.tile([BATCH, NJ], F32)
            nc.vector.reciprocal(rec, sm)
            oute = spool.tile([BATCH, NJ, Q], F32)
            nc.gpsimd.tensor_tensor(
                out=oute,
                in0=em,
                in1=rec.unsqueeze(2).broadcast_to([BATCH, NJ, Q]),
                op=ALU.mult,
            )
            nc.sync.dma_start(out=o_d[:, :, :], in_=oute)

    nc.compile()
    return nc


def _bf16_split(x):
    hi = x.astype(BF16_NP)
    lo = (x - hi.astype(np.float32)).astype(BF16_NP)
    return hi, lo


def _host_factors(wsl):
    """F18 [18, 128] bf16 and H18 [18, NKP, 2, 512] bf16 for one core.

    arg[j,k,m,l] = w[j,k] * (-(mv-lv)^2) = sum_r f_r(mv) * h_r(w, lv)
      f = [mv^2, mv, 1],  h = [-w, 2*w*lv, -w*lv^2]
    K rows ordered (t, c, r): t = split term (Fhi*Hhi, Flo*Hhi, Fhi*Hlo),
    c = k-half (F zeroed outside its c partition block), r = rank.
    """
    mv = (np.arange(Q, dtype=np.float32) / Q).astype(np.float32)
    lv = mv
    f = np.stack([mv * mv, mv, np.ones(Q, dtype=np.float32)]) / 7.0  # [3, 64]
    fhi, flo = _bf16_split(f)
    F18 = np.zeros((3, 2, 3, 128), dtype=BF16_NP)
    for c in range(2):
        sl = slice(c * 64, (c + 1) * 64)
        F18[0, c, :, sl] = fhi
        F18[1, c, :, sl] = flo
        F18[2, c, :, sl] = fhi
    F128 = np.zeros((128, 128), dtype=BF16_NP)
    F128[0:126] = np.tile(F18.reshape(18, 128), (7, 1))

    # h[r, c, j, kp, l] with w = wsl[j, kp + 64 c]
    wz = np.stack([wsl[:, 0:NKP], wsl[:, NKP:]], axis=0)  # [c, j, kp]
    h = np.empty((3, 2, NJ, NKP, Q), dtype=np.float32)
    h[0] = -wz[..., None] * np.ones(Q, dtype=np.float32)
    h[1] = 2.0 * wz[..., None] * lv
    h[2] = -wz[..., None] * (lv * lv)
    hhi, hlo = _bf16_split(h)
    H18 = np.zeros((3, 2, 3, NKP, 2, 8, Q), dtype=BF16_NP)
    for t, hv in ((0, hhi), (1, hhi), (2, hlo)):
        # hv [r, c, j, kp, l] -> [c, r, kp, jh, j8, l]
        H18[t] = (
            hv.reshape(3, 2, 2, 8, NKP, Q)
            .transpose(1, 0, 4, 2, 3, 5)
            .astype(BF16_NP)
        )
    return (
        np.ascontiguousarray(F128),
        np.ascontiguousarray(H18.reshape(18, NKP, 2, 512)),
    )


def kernel(P, weight, bias_abs, bias_q, lambda_abs, lambda_q):
    global _NC, LAST_RESULTS
    P = np.asarray(P, dtype=np.float32)
    PTH = np.ascontiguousarray(P.reshape(BATCH, 2, NKP, Q).transpose(1, 3, 2, 0))
    weight = np.asarray(weight, dtype=np.float32)
    bias_abs = np.asarray(bias_abs, dtype=np.float32)
    bias_q = np.asarray(bias_q, dtype=np.float32)
    lambda_abs = np.asarray(lambda_abs, dtype=np.float32)
    lambda_q = np.asarray(lambda_q, dtype=np.float32)

    if _NC is None:
        _NC = _build()

    s = (np.arange(Q, dtype=np.float32) / Q)[None, :]  # [1, 64]
    in_maps = []
    for c in range(N_CORES):
        jsl = slice(c * NJ, (c + 1) * NJ)
        F128, H18 = _host_factors(weight[jsl, :])
        Bm = -bias_q[jsl] * (s - lambda_q[jsl]) ** 2 - bias_abs[jsl] * np.abs(
            s - lambda_abs[jsl]
        )  # [16, 64]
        in_maps.append(
            {
                "PTH": PTH,
                "F128": F128,
                "H18": H18,
                "Bflat": np.ascontiguousarray(Bm.reshape(1, NJ * Q)),
            }
        )

    LAST_RESULTS = run_bass_kernel_spmd(_NC, in_maps, list(range(N_CORES)))
    return np.concatenate(
        [LAST_RESULTS.results[c]["out"] for c in range(N_CORES)], axis=1
    )
